# revision 5
# baseline (speedup 1.0000x reference)
"""Trainium2 Bass kernel v2 for the dense transformer block (LN1 -> MHA -> LN2 -> MLP).

Sharding: 8 cores = (batch 0..3) x (query half 0..1). The host rolls each
core's sequence so its 1024 query tokens always sit at positions 1024:2048
(K/V softmax is permutation-invariant over keys), so one program serves all
cores. Zero cross-core communication.

Dtypes: attention path fp8e4 end-to-end (XN, Wq/Wk/Wv/Wo, K/Q stores, probs,
V, ctx) with DoubleRow matmuls; MLP bf16; residuals/psum fp32.

Layout tricks:
 - Q/K projection output features are host-permuted so each head's 64-dim
   contraction becomes [32 partitions x 2 k-tiles] for DoubleRow scores.
 - V is stored [kv-part, kv-chunk, head, 80] with V in cols 0:64 and ones in
   col 64: the ctx matmul then yields both ctx (rows 0:64) and the softmax
   denominator (row 64) in one accumulation.
 - The block pipeline runs attention, LN2 and the MLP per 256-token block so
   PE (MLP) overlaps Act (softmax exp) across blocks.
"""

import sys
from contextlib import ExitStack

if '/opt/trn_rl_repo' not in sys.path:
    sys.path.insert(0, '/opt/trn_rl_repo')

import numpy as np
import ml_dtypes

import concourse.tile as tile
import concourse.mybir as mybir
from concourse import bacc
from concourse import bass_isa
from concourse.bass import ts
from concourse.bass import _add_dep_helper
from concourse.bass_utils import run_bass_kernel_spmd

P = 128
F32 = mybir.dt.float32
F32R = mybir.dt.float32r
BF16 = mybir.dt.bfloat16
F8 = mybir.dt.float8e4
AF = mybir.ActivationFunctionType
ALU = mybir.AluOpType
DR = mybir.MatmulPerfMode.DoubleRow
EPS = 1e-6

B, S, D, H, MLP = 4, 2048, 1024, 16, 4096
N_CORES = 8


def _qk_perm(Dm, Hh):
    """Feature perm so head h occupies partitions 32*(h%4).. of chunk-pair
    2*(h//4): feature (h, j*32+p) -> (2*(h//4)+j)*128 + (h%4)*32 + p."""
    perm = np.empty(Dm, np.int64)
    for f in range(Dm):
        h, r = divmod(f, 64)
        j, p = divmod(r, 32)
        perm[f] = (2 * (h // 4) + j) * 128 + (h % 4) * 32 + p
    return perm


def build_bass(T, Q, Dm, Hh, Mlp, n_cores, dbg=False):
    dh = Dm // Hh
    assert dh == 64
    n_dc = Dm // P          # 8 feature chunks
    n_tk = T // P           # 16 kv chunks
    TB = 512                # LN1 token block
    n_tb = T // TB
    QB = 256                # attention/MLP pipeline block
    n_qb = Q // QB          # 4
    n_mo = Mlp // P         # 32
    inv_d = 1.0 / Dm

    nc = bacc.Bacc("TRN2", target_bir_lowering=False, debug=False,
                   enable_asserts=False, num_devices=n_cores)

    def din(name, shape, dt):
        return nc.dram_tensor(name, shape, dt, kind="ExternalInput").ap()

    xT_d = din("xT", (Dm, T), F32)
    wq_d = din("wq8", (n_dc, P, n_dc, P), F8)
    wk_d = din("wk8", (n_dc, P, n_dc, P), F8)
    wv_d = din("wv8", (2, P, n_dc, 512), F8)
    wo_d = din("wo8", (n_dc, P, n_dc, P), F8)
    w1_d = din("w1t", (n_mo // 2, P, 2, n_dc, P), BF16)
    w2_d = din("w2t", (n_dc, P, n_mo, P), BF16)
    g1_d, be1_d = din("g1", (Dm,), F32), din("be1", (Dm,), F32)
    g2_d, be2_d = din("g2", (Dm,), F32), din("be2", (Dm,), F32)
    bq_d, bk_d = din("bqp", (Dm,), F32), din("bkp", (Dm,), F32)
    bo_d, b2_d = din("bo", (Dm,), F32), din("b2", (Dm,), F32)
    b1_d = din("b1", (Mlp,), F32)
    bv_d = din("bv16", (1, Dm), BF16)
    ones_d = din("ones16", (P, 1), BF16)
    onesr_d = din("ones_r", (P, 1), F32R)
    yT_d = nc.dram_tensor("yT", (Dm, Q), F32, kind="ExternalOutput").ap()
    dbg_d = {}
    if dbg:
        for nm, shape, dt in [("dXN", (Dm, T), F8), ("dKT", (Dm, T), F8),
                              ("dQT", (Dm, Q), F8), ("dVT", (P, n_tk * Hh * 80), F8),
                              ("dCT", (P, n_dc * 256), F8), ("dEX", (P, n_tk * 256), F8),
                              ("dXQ", (Dm, Q), F32), ("dXN2", (Dm, Q), BF16),
                              ("dY1", (P, n_mo * 256), BF16), ("dRB", (Hh, 256), F32),
                              ("dDN", (Hh, 256), F32)]:
            dbg_d[nm] = nc.dram_tensor(nm, shape, dt, kind="ExternalOutput").ap()

    with tile.TileContext(nc) as tc, ExitStack() as stack:
        if True:
            constp = stack.enter_context(tc.tile_pool(name="const", bufs=1))
            ones_h = constp.tile([P, 1], BF16)
            nc.sync.dma_start(ones_h[:], ones_d[:, :])
            ones_r = constp.tile([P, 1], F32R)
            nc.sync.dma_start(ones_r[:], onesr_d[:, :])
            eps_t = constp.tile([1, 1], F32)
            nc.vector.memset(eps_t[:], EPS)
            ones1 = constp.tile([1, P], BF16)
            nc.vector.memset(ones1[:], 1.0)
            negtwo = constp.tile([P, 1], F32)
            nc.vector.memset(negtwo[:], -4.5)
            bv_row = constp.tile([1, Dm], BF16)
            nc.sync.dma_start(bv_row[:, :], bv_d[:, :])

            def vec_tile(src, n, nm):
                t = constp.tile([P, n], F32, tag=nm, name=nm)
                nc.sync.dma_start(t[:], src.rearrange("(c p) -> p c", p=P))
                return t

            g1_t, be1_t = vec_tile(g1_d, n_dc, "g1"), vec_tile(be1_d, n_dc, "be1")
            g2_t, be2_t = vec_tile(g2_d, n_dc, "g2"), vec_tile(be2_d, n_dc, "be2")
            bq_t, bk_t = vec_tile(bq_d, n_dc, "bq"), vec_tile(bk_d, n_dc, "bk")
            bo_t, b2_t = vec_tile(bo_d, n_dc, "bo"), vec_tile(b2_d, n_dc, "b2")
            b1_t = vec_tile(b1_d, n_mo, "b1")

            def layernorm(src_fn, Tn, TBn, g_t, b_t, out_fn, ps_st, p_st, p_tmp,
                          final_on_act=False):
                """src_fn(dc, tb)->[P,TBn] f32 AP; out_fn(dc, tb)->dst AP."""
                order = [2, 0, 3, 1] if Tn // TBn == 4 else list(range(Tn // TBn))
                for tb in order:
                    if ps_st is None:
                        accm = p_st.tile([1, TBn], F32, tag="ln_accm")
                        accs = p_st.tile([1, TBn], F32, tag="ln_accs")
                        for dc in range(n_dc):
                            xc = src_fn(dc, tb)
                            xsq = p_tmp.tile([P, TBn], BF16, tag="ln_xsq")
                            nc.gpsimd.tensor_mul(xsq[:], xc, xc)
                            sx = p_tmp.tile([P, TBn], F32, tag="ln_arx")
                            nc.gpsimd.partition_all_reduce(
                                sx[:], xc, P, bass_isa.ReduceOp.add)
                            sq = p_tmp.tile([P, TBn], F32, tag="ln_arq")
                            nc.gpsimd.partition_all_reduce(
                                sq[:], xsq[:], P, bass_isa.ReduceOp.add)
                            if dc == 0:
                                nc.vector.tensor_copy(accm[:], sx[0:1, :])
                                nc.vector.tensor_copy(accs[:], sq[0:1, :])
                            else:
                                nc.vector.tensor_add(accm[:], accm[:],
                                                     sx[0:1, :])
                                nc.vector.tensor_add(accs[:], accs[:],
                                                     sq[0:1, :])
                        ps_m, ps_s = accm[:], accs[:]
                    else:
                        ps_ms = ps_st.tile([1, 2, TBn], F32, tag="ps_stat")
                        ps_m, ps_s = ps_ms[:, 0, :], ps_ms[:, 1, :]
                        for dc in range(n_dc):
                            st, sp = (dc == 0), (dc == n_dc - 1)
                            xc = src_fn(dc, tb)
                            nc.tensor.matmul(ps_m, ones_r[:], xc.bitcast(F32R),
                                             start=st, stop=sp)
                            xsq = p_tmp.tile([P, TBn], BF16, tag="ln_xsq")
                            nc.gpsimd.tensor_mul(xsq[:], xc, xc)
                            nc.tensor.matmul(ps_s, ones_h[:], xsq[:],
                                             start=st, stop=sp)
                    mean = p_st.tile([1, TBn], F32, tag="ln_mean")
                    nc.vector.tensor_scalar_mul(mean[:], ps_m, inv_d)
                    var = p_st.tile([1, TBn], F32, tag="ln_var")
                    # var = ps_s*inv_d - mean^2 (+eps)
                    m2 = p_st.tile([1, TBn], F32, tag="ln_m2")
                    nc.vector.tensor_mul(m2[:], mean[:], mean[:])
                    nc.vector.tensor_scalar(var[:], ps_s, inv_d, EPS,
                                            ALU.mult, ALU.add)
                    nc.vector.tensor_sub(var[:], var[:], m2[:])
                    lnv = p_st.tile([1, TBn], F32, tag="ln_lnv")
                    nc.scalar.activation(lnv[:], var[:], AF.Ln)
                    rstd = p_st.tile([1, TBn], BF16, tag="ln_rstd")
                    nc.scalar.activation(rstd[:], lnv[:], AF.Exp, scale=-0.5)
                    mean_bc = p_tmp.tile([P, TBn], F32, tag="ln_meanbc")
                    rstd_bc = p_tmp.tile([P, TBn], BF16, tag="ln_rstdbc")
                    nc.gpsimd.partition_broadcast(mean_bc[:], mean[:])
                    nc.gpsimd.partition_broadcast(rstd_bc[:], rstd[:])
                    for dc in range(n_dc):
                        t0 = p_tmp.tile([P, TBn], BF16, tag="ln_t0")
                        nc.vector.tensor_sub(t0[:], src_fn(dc, tb), mean_bc[:])
                        t1 = p_tmp.tile([P, TBn], BF16, tag="ln_t1")
                        nc.vector.tensor_mul(t1[:], t0[:], rstd_bc[:])
                        if final_on_act:
                            nc.scalar.activation(out_fn(dc, tb), t1[:],
                                                 AF.Identity,
                                                 bias=b_t[:, dc:dc + 1],
                                                 scale=g_t[:, dc:dc + 1])
                        else:
                            nc.vector.tensor_scalar(out_fn(dc, tb), t1[:],
                                                    g_t[:, dc:dc + 1],
                                                    b_t[:, dc:dc + 1],
                                                    ALU.mult, ALU.add)

            p_xq = stack.enter_context(tc.tile_pool(name="p_xq", bufs=1))
            XQ = p_xq.tile([P, n_dc, Q], F32)     # query-half x, then h2
            p_kv = stack.enter_context(tc.tile_pool(name="p_kv", bufs=1))
            KT = p_kv.tile([P, n_dc, T], F8)
            QT = p_kv.tile([P, n_dc, Q], F8)
            VTe = p_kv.tile([P, n_tk, Hh, 80], F8)

            def phase_a():
                # ---------- LN1 + QKV/V projections ----------
                with ExitStack() as es:
                    pool = lambda *a, **k: es.enter_context(tc.tile_pool(*a, **k))
                    p_xa = pool(name="p_xa", bufs=1)
                    p_xn = pool(name="p_xn", bufs=1)
                    p_lt = pool(name="p_lt", bufs=3)
                    p_ls = pool(name="p_ls", bufs=2)
                    p_wst = pool(name="p_wst", bufs=2)
                    ps_st = pool(name="ps_st", bufs=1, space="PSUM")
                    ps_mm = pool(name="ps_mm", bufs=3, space="PSUM")

                    XA = p_xa.tile([P, n_dc, T], F32)  # LN1 source (f32r-typed)
                    for tb in [2, 0, 3, 1]:
                        for dc in range(n_dc):
                            nc.sync.dma_start(
                                XA[:, dc, ts(tb, TB)].bitcast(F32R),
                                xT_d[ts(dc, P), ts(tb, TB)].bitcast(F32R))
                    for dc in range(n_dc):
                        nc.sync.dma_start(XQ[:, dc, :], xT_d[ts(dc, P), T // 2:T])
                    XN = p_xn.tile([P, n_dc, T], F8)

                    def lsrc(dc, tb):
                        return XA[:, dc, ts(tb, TB)]

                    layernorm(lsrc, T, TB, g1_t, be1_t,
                              lambda dc, tb: XN[:, dc, ts(tb, TB)],
                              ps_st, p_ls, p_lt, final_on_act=True)

                    # ones column for the fused softmax denominator
                    nc.gpsimd.memset(VTe[:, :, :, 64], 1.0)

                    # Q first (queries ready first), then K in kv-need order
                    for mo in range(n_dc):
                        wt = p_wst.tile([P, n_dc, P], F8, tag="wkq")
                        nc.sync.dma_start(wt[:], wq_d[mo])
                        for qb in range(2):
                            ps = ps_mm.tile([P, TB], F32, tag="ps_kq")
                            for d in range(n_dc // 2):
                                nc.tensor.matmul(
                                    ps[:], wt[:, 2 * d:2 * d + 2, :],
                                    XN[:, 2 * d:2 * d + 2,
                                       T // 2 + qb * TB:T // 2 + (qb + 1) * TB],
                                    start=(d == 0), stop=(d == 3),
                                    perf_mode=DR)
                            nc.scalar.activation(QT[:, mo, ts(qb, TB)],
                                                 ps[:], AF.Identity,
                                                 bias=bq_t[:, mo:mo + 1])
                    for tb in [0, 3, 1, 2]:
                        for mo in range(n_dc):
                            wt = p_wst.tile([P, n_dc, P], F8, tag="wkq")
                            nc.sync.dma_start(wt[:], wk_d[mo])
                            ps = ps_mm.tile([P, TB], F32, tag="ps_kq")
                            for d in range(n_dc // 2):
                                nc.tensor.matmul(
                                    ps[:], wt[:, 2 * d:2 * d + 2, :],
                                    XN[:, 2 * d:2 * d + 2, ts(tb, TB)],
                                    start=(d == 0), stop=(d == 3),
                                    perf_mode=DR)
                            nc.scalar.activation(KT[:, mo, ts(tb, TB)],
                                                 ps[:], AF.Identity,
                                                 bias=bk_t[:, mo:mo + 1])
                    # V: out [token, feature]; bias via ones x bv matmul
                    for no in range(2):
                        wvt = p_wst.tile([P, n_dc, 512], F8, tag="wv")
                        nc.sync.dma_start(wvt[:], wv_d[no])
                        for to in range(n_tk):
                            ps = ps_mm.tile([P, 8, 64], F32, tag="ps_v")
                            for d in range(n_dc // 2):
                                nc.tensor.matmul(
                                    ps[:], XN[:, 2 * d:2 * d + 2, ts(to, P)],
                                    wvt[:, 2 * d:2 * d + 2, :],
                                    start=(d == 0), stop=False,
                                    perf_mode=DR)
                            nc.tensor.matmul(ps[:], ones1[:, :],
                                             bv_row[:, ts(no, 512)],
                                             start=False, stop=True)
                            nc.scalar.activation(
                                VTe[:, to, 8 * no:8 * no + 8, 0:64], ps[:],
                                AF.Identity)

                    if dbg:
                        for dc in range(n_dc):
                            nc.sync.dma_start(dbg_d["dXN"][ts(dc, P), :], XN[:, dc, :])
                            nc.sync.dma_start(dbg_d["dKT"][ts(dc, P), :], KT[:, dc, :])
                            nc.sync.dma_start(dbg_d["dQT"][ts(dc, P), :], QT[:, dc, :])
                        nc.sync.dma_start(dbg_d["dVT"][:, :],
                                          VTe[:].rearrange("p a b c -> p (a b c)"))

            def phase_bd():
                # ---------- per-block attention + LN2 + MLP ----------
                with ExitStack() as es:
                    pool = lambda *a, **k: es.enter_context(tc.tile_pool(*a, **k))
                    p_wo = pool(name="p_wo", bufs=1)
                    p_mlp = pool(name="p_mlp", bufs=1)
                    p_exp = pool(name="p_exp", bufs=3)
                    p_ct = pool(name="p_ct", bufs=2)
                    p_rb = pool(name="p_rb", bufs=2)
                    p_lt2 = pool(name="p_lt2", bufs=2)
                    p_ls2 = pool(name="p_ls2", bufs=2)
                    p_y1 = pool(name="p_y1", bufs=2)
                    p_w1 = pool(name="p_w1", bufs=2)
                    p_w2 = pool(name="p_w2", bufs=2)
                    p_out = pool(name="p_out", bufs=4)
                    ps_sc = pool(name="ps_sc", bufs=2, space="PSUM")
                    ps_ctx = pool(name="ps_ctx", bufs=1, space="PSUM")
                    ps_wo = pool(name="ps_wo", bufs=1, space="PSUM")
                    ps_f = pool(name="ps_f", bufs=2, space="PSUM")

                    wot = [p_wo.tile([P, n_dc, P], F8, tag=f"wo{m}", name="wo")
                           for m in range(n_dc)]
                    for mo in range(n_dc):
                        nc.sync.dma_start(wot[mo][:], wo_d[mo])
                    XN2 = p_mlp.tile([P, n_dc, Q], BF16)
                    ct_tiles = {}
                    last_exp = {}
                    dbg_ex = {}

                    def attention_block(blk):
                        qsl = ts(blk, QB)
                        CT = p_ct.tile([P, n_dc, QB], F8, tag="ct")
                        ct_tiles[blk] = CT
                        for h in range(Hh):
                            p0 = 32 * (h % 4)
                            cb = 2 * (h // 4)
                            ex = p_exp.tile([P, n_tk, QB], F8, tag="exp")
                            if dbg and blk == 1 and h == 0:
                                dbg_ex[0] = ex
                            for g in [0, 3, 1, 2]:
                                sc4 = ps_sc.tile([P, 4, QB], F32, tag="sc")
                                for i in range(4):
                                    kv = 4 * g + i
                                    nc.tensor.matmul(
                                        sc4[:, i, :],
                                        KT[p0:p0 + 32, cb:cb + 2, ts(kv, P)],
                                        QT[p0:p0 + 32, cb:cb + 2, qsl],
                                        start=True, stop=True, perf_mode=DR,
                                        tile_position=(p0, 0))
                                ei = nc.scalar.activation(
                                    ex[:, 4 * g:4 * g + 4, :], sc4[:],
                                    AF.Exp, bias=negtwo[:, :], scale=0.125)
                                if g == 3:
                                    last_exp.setdefault(blk, []).append(ei)
                            ps_c = ps_ctx.tile([P, QB], F32, tag="ctx")
                            for kc in range(n_tk // 2):
                                nc.tensor.matmul(
                                    ps_c[0:65, :],
                                    VTe[:, 2 * kc:2 * kc + 2, h, 0:65],
                                    ex[:, 2 * kc:2 * kc + 2, :],
                                    start=(kc == 0), stop=(kc == 7),
                                    perf_mode=DR)
                            rbc = p_rb.tile([1, QB], F32, tag="rbc")
                            nc.vector.reciprocal(rbc[:], ps_c[64:65, :])
                            if dbg and blk == 1:
                                nc.sync.dma_start(dbg_d["dRB"][h:h + 1, :], rbc[:])
                            rbb = p_rb.tile([64, QB], F32, tag="rbb")
                            nc.gpsimd.partition_broadcast(rbb[:], rbc[:])
                            nc.vector.tensor_mul(
                                CT[64 * (h % 2):64 * (h % 2) + 64, h // 2, :],
                                ps_c[0:64, :], rbb[:])

                    def tail_block(blk):
                        qsl = ts(blk, QB)
                        CT = ct_tiles.pop(blk)
                        for mo in range(n_dc):
                            ps = ps_wo.tile([P, QB], F32, tag="ps_w")
                            for d in range(n_dc // 2):
                                nc.tensor.matmul(
                                    ps[:], wot[mo][:, 2 * d:2 * d + 2, :],
                                    CT[:, 2 * d:2 * d + 2, :],
                                    start=(d == 0), stop=(d == 3),
                                    perf_mode=DR)
                            t = p_rb.tile([P, QB], F32, tag="wod")
                            nc.vector.tensor_scalar(t[:], ps[:],
                                                    bo_t[:, mo:mo + 1], None,
                                                    ALU.add)
                            nc.vector.tensor_add(XQ[:, mo, qsl], t[:],
                                                 XQ[:, mo, qsl])
                        layernorm(lambda dc, tb, _q=qsl: XQ[:, dc, _q],
                                  QB, QB, g2_t, be2_t,
                                  lambda dc, tb, _q=qsl: XN2[:, dc, _q],
                                  None, p_ls2, p_lt2)
                        Y1 = p_y1.tile([P, n_mo, QB], BF16, tag="y1")
                        if blk == n_qb - 1:
                            # tail block: no later exps -> gelu straight from
                            # psum, using the now-idle scores psum pool
                            for mq in range(n_mo // 4):
                                wta = p_w1.tile([P, 2, n_dc, P], BF16, tag="w1")
                                nc.sync.dma_start(wta[:], w1_d[2 * mq])
                                wtb = p_w1.tile([P, 2, n_dc, P], BF16, tag="w1")
                                nc.sync.dma_start(wtb[:], w1_d[2 * mq + 1])
                                psf = ps_sc.tile([P, 4, QB], F32, tag="sc")
                                for i in range(4):
                                    wt = (wta, wtb)[i // 2]
                                    for dc in range(n_dc):
                                        nc.tensor.matmul(psf[:, i, :],
                                                         wt[:, i % 2, dc, :],
                                                         XN2[:, dc, qsl],
                                                         start=(dc == 0),
                                                         stop=(dc == n_dc - 1))
                                for i in range(4):
                                    nc.scalar.activation(
                                        Y1[:, 4 * mq + i, :], psf[:, i, :],
                                        AF.Gelu,
                                        bias=b1_t[:, 4 * mq + i:4 * mq + i + 1])
                        else:
                            for mp in range(n_mo // 2):
                                wt = p_w1.tile([P, 2, n_dc, P], BF16, tag="w1")
                                nc.sync.dma_start(wt[:], w1_d[mp])
                                for i in range(2):
                                    psf = ps_f.tile([P, QB], F32, tag="ps_f")
                                    for dc in range(n_dc):
                                        nc.tensor.matmul(psf[:],
                                                         wt[:, i, dc, :],
                                                         XN2[:, dc, qsl],
                                                         start=(dc == 0),
                                                         stop=(dc == n_dc - 1))
                                    nc.vector.tensor_scalar(
                                        Y1[:, 2 * mp + i, :], psf[:],
                                        b1_t[:, 2 * mp + i:2 * mp + i + 1],
                                        None, ALU.add)
                            gates = last_exp.get(blk + 1, [])
                            prev_g = None
                            for m8 in range(n_mo // 8):
                                gi = nc.scalar.activation(
                                    Y1[:, 8 * m8:8 * m8 + 8, :],
                                    Y1[:, 8 * m8:8 * m8 + 8, :], AF.Gelu)
                                for gate in gates:
                                    _add_dep_helper(gi.ins, gate.ins, sync=True,
                                                    reason="batch gelus after exps")
                                if prev_g is not None:
                                    _add_dep_helper(gi.ins, prev_g.ins,
                                                    sync=True,
                                                    reason="contiguous gelu batch")
                                prev_g = gi
                        for mo2 in range(n_dc):
                            wt = p_w2.tile([P, n_mo, P], BF16, tag="w2")
                            nc.sync.dma_start(wt[:], w2_d[mo2])
                            ps = ps_f.tile([P, QB], F32, tag="ps_f")
                            for k in range(n_mo):
                                nc.tensor.matmul(ps[:], wt[:, k, :],
                                                 Y1[:, k, :],
                                                 start=(k == 0),
                                                 stop=(k == n_mo - 1))
                            ot = p_out.tile([P, QB], F32, tag="out")
                            nc.vector.tensor_scalar(ot[:], ps[:],
                                                    b2_t[:, mo2:mo2 + 1], None,
                                                    ALU.add)
                            nc.vector.tensor_add(ot[:], ot[:], XQ[:, mo2, qsl])
                            nc.sync.dma_start(yT_d[ts(mo2, P), qsl], ot[:])

                    # depth-2 pipeline: keep next block's scores ahead of
                    # the previous block's MLP in the PE queue
                    attention_block(0)
                    attention_block(1)
                    for blk in range(2, n_qb):
                        tail_block(blk - 2)
                        attention_block(blk)
                    tail_block(n_qb - 2)
                    tail_block(n_qb - 1)
                    if dbg:
                        for dc in range(n_dc):
                            nc.sync.dma_start(dbg_d["dXQ"][ts(dc, P), :], XQ[:, dc, :])
                            nc.sync.dma_start(dbg_d["dXN2"][ts(dc, P), :], XN2[:, dc, :])

            phase_a()
            phase_bd()
    nc.compile()
    return nc


_NC_CACHE = {}


def _get_nc(T, Q, Dm, Hh, Mlp, n_cores):
    key = (T, Q, Dm, Hh, Mlp, n_cores)
    if key not in _NC_CACHE:
        _NC_CACHE[key] = build_bass(T, Q, Dm, Hh, Mlp, n_cores)
    return _NC_CACHE[key]


def make_in_maps(inputs, n_cores):
    x = np.asarray(inputs["x"], np.float32)
    Bq, Sq, Dq = x.shape
    Qtok = Sq * Bq // n_cores
    bf = ml_dtypes.bfloat16
    f8 = mybir.dt.np(F8)
    n_dc, n_mo = Dq // P, MLP // P

    perm = _qk_perm(Dq, H)
    Wq = np.asarray(inputs["Wq"], np.float32)
    Wk = np.asarray(inputs["Wk"], np.float32)
    Wq_p = np.empty_like(Wq); Wq_p[:, perm] = Wq
    Wk_p = np.empty_like(Wk); Wk_p[:, perm] = Wk
    bq_p = np.empty(Dq, np.float32); bq_p[perm] = np.asarray(inputs["bq"], np.float32)
    bk_p = np.empty(Dq, np.float32); bk_p[perm] = np.asarray(inputs["bk"], np.float32)

    def wblk(W):  # (D, D) -> (n_dc, P, n_dc, P): [mo][p][c][m]
        return np.ascontiguousarray(
            W.reshape(n_dc, P, n_dc, P).transpose(2, 1, 0, 3))

    W1 = np.asarray(inputs["W1"], np.float32)
    W2 = np.asarray(inputs["W2"], np.float32)
    # w1t: (16, P, 2, n_dc, P): [mp][p][i][c][m], mo = 2*mp+i
    w1t = W1.reshape(n_dc, P, n_mo, P).transpose(2, 1, 0, 3)  # [mo][p][c][m]
    w1t = np.ascontiguousarray(
        w1t.reshape(n_mo // 2, 2, P, n_dc, P).transpose(0, 2, 1, 3, 4))
    w2t = np.ascontiguousarray(
        W2.reshape(n_mo, P, n_dc, P).transpose(2, 1, 0, 3))  # [mo2][p][k][m]
    wv = np.asarray(inputs["Wv"], np.float32)
    wvt = np.ascontiguousarray(
        wv.reshape(n_dc, P, 2, 512).transpose(2, 1, 0, 3))   # [no][p][c][f]

    shared = {
        "wq8": wblk(Wq_p).astype(f8),
        "wk8": wblk(Wk_p).astype(f8),
        "wv8": wvt.astype(f8),
        "wo8": wblk(np.asarray(inputs["Wo"], np.float32)).astype(f8),
        "w1t": w1t.astype(bf),
        "w2t": w2t.astype(bf),
        "g1": np.asarray(inputs["ln1_g"], np.float32),
        "be1": np.asarray(inputs["ln1_b"], np.float32),
        "g2": np.asarray(inputs["ln2_g"], np.float32),
        "be2": np.asarray(inputs["ln2_b"], np.float32),
        "bqp": bq_p, "bkp": bk_p,
        "bo": np.asarray(inputs["bo"], np.float32),
        "b1": np.asarray(inputs["b1"], np.float32),
        "b2": np.asarray(inputs["b2"], np.float32),
        "bv16": np.asarray(inputs["bv"], np.float32).reshape(1, -1).astype(bf),
        "ones16": np.ones((P, 1), bf),
        "ones_r": np.ones((P, 1), np.float32),
    }
    in_maps = []
    for c in range(n_cores):
        b = c // (n_cores // Bq)
        qhalf = c % (n_cores // Bq)
        if qhalf == 0:
            view = np.concatenate([x[b, Qtok:], x[b, :Qtok]], axis=0)
        else:
            view = x[b]
        m = dict(shared)
        m["xT"] = np.ascontiguousarray(view.T)
        in_maps.append(m)
    return in_maps, Qtok


def kernel(**inputs):
    x = np.asarray(inputs["x"], np.float32)
    Bq, Sq, Dq = x.shape
    in_maps, Qtok = make_in_maps(inputs, N_CORES)
    nc = _get_nc(Sq, Qtok, Dq, H, MLP, N_CORES)
    res = run_bass_kernel_spmd(nc, in_maps, core_ids=list(range(N_CORES)))
    out = np.empty((Bq, Sq, Dq), np.float32)
    per_b = N_CORES // Bq
    for c in range(N_CORES):
        b = c // per_b
        qoff = (c % per_b) * Qtok
        out[b, qoff:qoff + Qtok, :] = res.results[c]["yT"].T
    return out
